# revision 27
# baseline (speedup 1.0000x reference)
"""Trainium2 Bass kernel for nn_AutoInt (AutoInt CTR model).

Model: 26 sparse fields (one-hot embedding lookup, VOC=11) + 13 dense fields
(per-field Linear(1,8)) -> h [B, 39, 8]; 3 multi-head (H=2, D=4) interacting
layers (self-attention over the 39 feature fields, residual proj, relu);
outputs emb = h.reshape(B, 312) and out = softmax(emb @ fc_W + fc_b).

Strategy (pure data parallel over 8 cores, batch-sharded):
  - host: transpose each core's x shard to [39, b_loc]; build layout-only
    constants (block-diagonal weights, one-hot selector/compare tables).
  - device, per 128-sample tile (sample-major h [128, 312] throughout):
      embedding: replicate-rows matmul -> is_equal -> one-hot @ blockdiag
      tables (PE), dense part fused into the same PSUM bank.
      per layer: PE-transpose h -> h^T groups; projections with h^T as the
      matmul *stationary* and block-diag Wq|Wk|Wv|Wres as moving -> q,k,v,res
      sample-major; scores/softmax/att@v via DVE broadcast-view tensor ops +
      ACT exp; residual + relu.
      head: h^T groups @ fc_W accumulate -> logits; batched softmax at end.
"""

import os
from contextlib import ExitStack

import numpy as np

import concourse.bass as bass
import concourse.tile as tile
from concourse import bacc, mybir
from concourse.bass_utils import run_bass_kernel_spmd

F32 = mybir.dt.float32
AF = mybir.ActivationFunctionType
OP = mybir.AluOpType


def _dep(a, b, sync=True, reason="psum-group order"):
    """Order instruction b after a (BassInstruction wrappers)."""
    tile.add_dep_helper(b.ins, a.ins, sync=sync, reason=reason)

N_SPARSE, N_DENSE = 26, 13
F, E, H, D, NL, VOC = 39, 8, 2, 4, 3, 11
N_CORES = 8


# ---------------------------------------------------------------- host consts
def _build_consts(emb_tables, dense_W, dense_b, Wq, Wk, Wv, Wres, fc_W, fc_b):
    f32 = np.float32
    emb_tables = np.asarray(emb_tables, f32)
    dense_W = np.asarray(dense_W, f32)
    dense_b = np.asarray(dense_b, f32)
    fc_W = np.asarray(fc_W, f32)
    fc_b = np.asarray(fc_b, f32)

    # block-diag projection weights, all layers side by side: [104, 12*104]
    eye13 = np.eye(13, dtype=f32)
    wbd = np.concatenate(
        [
            np.kron(eye13, np.asarray(W[l], f32))
            for l in range(NL)
            for W in (Wq, Wk, Wv, Wres)
        ],
        axis=1,
    )

    # one-hot row-replication selector R [39, 287]
    R = np.zeros((39, 287), f32)
    for g in range(2):
        for fl in range(11):
            for v in range(VOC):
                R[g * 11 + fl, 121 * g + 11 * fl + v] = 1.0
    for fl in range(4):
        for v in range(VOC):
            R[22 + fl, 242 + 11 * fl + v] = 1.0

    # compare values [121, 2] (col 0: groups 0/1; col 1: group 2 + ones-row)
    vals = np.zeros((121, 2), f32)
    vals[:, 0] = np.tile(np.arange(VOC, dtype=f32), 11)
    vals[:44, 1] = np.tile(np.arange(VOC, dtype=f32), 4)
    # row 44 col 1 stays 0.0 -> is_equal(0, 0) = 1 = the "ones" row

    # embedding tables, block diagonal per group
    tbd01 = np.zeros((121, 176), f32)
    for g in range(2):
        for fl in range(11):
            tbd01[11 * fl : 11 * fl + 11, 88 * g + 8 * fl : 88 * g + 8 * fl + 8] = (
                emb_tables[g * 11 + fl]
            )
    tbd2e = np.zeros((45, 136), f32)
    for fl in range(4):
        tbd2e[11 * fl : 11 * fl + 11, 8 * fl : 8 * fl + 8] = emb_tables[22 + fl]
    tbd2e[44, 32:136] = dense_b.reshape(104)

    # dense weight diag [13, 104]
    wdd = np.zeros((13, 104), f32)
    for fl in range(13):
        wdd[fl, 8 * fl : 8 * fl + 8] = dense_W[fl]

    idn = np.eye(128, dtype=f32)
    fcw = np.ascontiguousarray(
        np.concatenate([fc_W.reshape(3, 104, 2)[g] for g in range(3)], axis=1)
    )  # [104, 6]
    fcb = np.ascontiguousarray(np.broadcast_to(fc_b.reshape(1, 2), (128, 2)))

    return {
        "wbd": np.ascontiguousarray(wbd),
        "rmat": R,
        "vals": vals,
        "tbd01": tbd01,
        "tbd2e": tbd2e,
        "wdd": wdd,
        "idn": idn,
        "fcw": fcw,
        "fcb": fcb,
    }


CONST_SHAPES = {
    "wbd": (104, 1248),
    "rmat": (39, 287),
    "vals": (121, 2),
    "tbd01": (121, 176),
    "tbd2e": (45, 136),
    "wdd": (13, 104),
    "idn": (128, 128),
    "fcw": (104, 6),
    "fcb": (128, 2),
}


# ---------------------------------------------------------------- device build
def build_nc(b_loc):
    n_tiles = b_loc // 128
    assert b_loc % 128 == 0
    nc = bacc.Bacc(
        "TRN2", target_bir_lowering=False, debug=False, num_devices=N_CORES
    )
    xt_d = nc.dram_tensor("xt", [39, b_loc], F32, kind="ExternalInput").ap()
    cd = {
        name: nc.dram_tensor(name, list(shape), F32, kind="ExternalInput").ap()
        for name, shape in CONST_SHAPES.items()
    }
    emb_d = nc.dram_tensor("emb", [b_loc, 312], F32, kind="ExternalOutput").ap()
    outp_d = nc.dram_tensor(
        "outp", [128, 2 * n_tiles], F32, kind="ExternalOutput"
    ).ap()

    with tile.TileContext(nc) as tc, ExitStack() as ctx:
        cpool = ctx.enter_context(tc.tile_pool(name="consts", bufs=1))
        cs = {}
        for name, shape in CONST_SHAPES.items():
            t = cpool.tile(list(shape), F32, tag=name)
            nc.sync.dma_start(t[:], cd[name][:])
            cs[name] = t
        xt = cpool.tile([39, b_loc], F32, tag="xt")
        nc.sync.dma_start(xt[:], xt_d[:])
        xdense = cpool.tile([13, b_loc], F32, tag="xdense")
        nc.sync.dma_start(xdense[:], xt_d[26:39, :])
        logits = cpool.tile([128, 2 * n_tiles], F32, tag="logits")

        hpool = ctx.enter_context(tc.tile_pool(name="h", bufs=6))
        htpool = ctx.enter_context(tc.tile_pool(name="ht", bufs=3))
        ohpool = ctx.enter_context(tc.tile_pool(name="oh", bufs=3))
        qkvr = ctx.enter_context(tc.tile_pool(name="qkvr", bufs=3))
        spool = ctx.enter_context(tc.tile_pool(name="sco", bufs=2))
        p2pool = ctx.enter_context(tc.tile_pool(name="p2", bufs=2))
        smpool = ctx.enter_context(tc.tile_pool(name="small", bufs=3))
        psA = ctx.enter_context(tc.tile_pool(name="psA", bufs=2, space="PSUM"))
        psB = ctx.enter_context(tc.tile_pool(name="psB", bufs=2, space="PSUM"))

        def emit_embed(ti):
            """One-hot embedding + dense features via PE -> h [128, 312]."""
            s0 = ti * 128
            pB = psB.tile([128, 512], F32, tag="psB")
            xcol = xt[:, s0 : s0 + 128]
            rmm1 = nc.tensor.matmul(
                pB[0:121, 0:128], cs["rmat"][:, 0:121], xcol, start=True, stop=False
            )
            rmm2 = nc.tensor.matmul(
                pB[0:45, 256:384], cs["rmat"][:, 242:287], xcol,
                start=False, stop=False,
            )
            rmm3 = nc.tensor.matmul(
                pB[0:121, 128:256], cs["rmat"][:, 121:242], xcol,
                start=False, stop=True,
            )
            _dep(rmm1, rmm2, sync=False)
            _dep(rmm2, rmm3, sync=False)
            oh = ohpool.tile([121, 384], F32)
            eqs = []
            for sl, vcol in (
                ((slice(0, 121), slice(0, 128)), 0),
                ((slice(0, 121), slice(128, 256)), 0),
                ((slice(0, 45), slice(256, 384)), 1),
            ):
                nr = sl[0].stop
                eq = nc.vector.tensor_scalar(
                    out=oh[sl[0], sl[1]],
                    in0=pB[sl[0], sl[1]],
                    scalar1=cs["vals"][0:nr, vcol : vcol + 1],
                    scalar2=None,
                    op0=OP.is_equal,
                )
                _dep(rmm3, eq)
                eqs.append(eq)
            pA = psA.tile([128, 1536], F32, tag="psA")
            em1 = nc.tensor.matmul(
                pA[:, 0:88], oh[0:121, 0:128], cs["tbd01"][:, 0:88],
                start=True, stop=False,
            )
            em2 = nc.tensor.matmul(
                pA[:, 88:176], oh[0:121, 128:256], cs["tbd01"][:, 88:176],
                start=False, stop=False,
            )
            em3 = nc.tensor.matmul(
                pA[:, 176:312], oh[0:45, 256:384], cs["tbd2e"][:],
                start=False, stop=False,
            )
            em4 = nc.tensor.matmul(
                pA[:, 208:312], xdense[:, s0 : s0 + 128], cs["wdd"][:],
                start=False, stop=True,
            )
            _dep(em1, em2, sync=False)
            _dep(em2, em3, sync=False)
            _dep(em3, em4, sync=False)
            h_sb = hpool.tile([128, 312], F32, tag="h")
            hcp = nc.scalar.copy(h_sb[:], pA[:, 0:312])
            _dep(em4, hcp)
            return h_sb

        def emit_transposes(h_sb):
            """PE-transpose h into h^T groups [104, 384] (PE + ACT only)."""
            pT = psB.tile([128, 512], F32, tag="psB")
            tms = []
            for g in range(3):
                tm = nc.tensor.matmul(
                    pT[0:104, 128 * g : 128 * (g + 1)],
                    h_sb[:, 104 * g : 104 * (g + 1)],
                    cs["idn"][:],
                    is_transpose=True,
                    start=(g == 0),
                    stop=(g == 2),
                )
                if tms:
                    _dep(tms[-1], tm, sync=False)
                tms.append(tm)
            ht = htpool.tile([104, 384], F32)
            for g in range(3):
                c = nc.scalar.copy(
                    ht[:, 128 * g : 128 * (g + 1)], pT[0:104, 128 * g : 128 * (g + 1)]
                )
                _dep(tms[-1], c)
            return ht, pT, tms

        def emit_proj(h_sb, li):
            """Projections q,k,v,res sample-major [128, 312] (PE + ACT only)."""
            ht, _, _ = emit_transposes(h_sb)
            pP = psA.tile([128, 1536], F32, tag="psA")
            for g in range(3):
                nc.tensor.matmul(
                    pP[:, 512 * g : 512 * g + 416],
                    ht[:, 128 * g : 128 * (g + 1)],
                    cs["wbd"][:, 416 * li : 416 * (li + 1)],
                    start=True,
                    stop=True,
                )
            qs = qkvr.tile([128, 312], F32, tag="q")
            ks = qkvr.tile([128, 312], F32, tag="k")
            vs = qkvr.tile([128, 312], F32, tag="v")
            rs = qkvr.tile([128, 312], F32, tag="r")
            pview = pP[:].rearrange("p (g c) -> p g c", g=3, c=512)
            for pi, dst in enumerate((qs, ks, vs, rs)):
                src = pview[:, :, 104 * pi : 104 * (pi + 1)]
                dstv = dst[:].rearrange("p (g c) -> p g c", g=3, c=104)
                nc.scalar.copy(dstv, src)
            return qs, ks, vs, rs

        def emit_att(qs, ks, vs, rs):
            """Scores/softmax/att@v/residual on DVE (+ ACT exp, relu)."""
            q4 = qs[:].rearrange("p (i h d) -> p h i d", i=39, h=2, d=4)
            k4 = ks[:].rearrange("p (j h d) -> p h j d", j=39, h=2, d=4)
            v4 = vs[:].rearrange("p (j h d) -> p h j d", j=39, h=2, d=4)
            Sh = []
            for hi in range(2):
                S = spool.tile([128, 1521], F32, tag=f"S{hi}")
                tmp = spool.tile([128, 1521], F32, tag=f"tmp{hi}")
                S3 = S[:].rearrange("p (i j) -> p i j", i=39, j=39)
                t3 = tmp[:].rearrange("p (i j) -> p i j", i=39, j=39)
                for di in range(4):
                    dst = S3 if di == 0 else t3
                    qv = q4[:, hi, :, di].unsqueeze(2).broadcast_to((128, 39, 39))
                    kv = k4[:, hi, :, di].unsqueeze(1).broadcast_to((128, 39, 39))
                    nc.vector.tensor_tensor(out=dst, in0=kv, in1=qv, op=OP.mult)
                    if di > 0:
                        nc.vector.tensor_tensor(
                            out=S[:], in0=S[:], in1=tmp[:], op=OP.add
                        )
                Sh.append(S)
            Gh, rech = [], []
            for hi in range(2):
                G = spool.tile([128, 1521], F32, tag=f"G{hi}")
                nc.scalar.activation(G[:], Sh[hi][:], AF.Exp)
                den = smpool.tile([128, 39], F32, tag=f"den{hi}")
                nc.vector.tensor_reduce(
                    den[:],
                    G[:].rearrange("p (i j) -> p i j", i=39, j=39),
                    axis=mybir.AxisListType.X,
                    op=OP.add,
                )
                rec = smpool.tile([128, 39], F32, tag=f"rec{hi}")
                nc.vector.reciprocal(rec[:], den[:])
                Gh.append(G)
                rech.append(rec)
            run_t = smpool.tile([128, 312], F32, tag="run")
            ru4 = run_t[:].rearrange("p (i h d) -> p i h d", i=39, h=2, d=4)
            for hi in range(2):
                P2 = p2pool.tile([128, 6084], F32, tag="P2")
                P24 = P2[:].rearrange("p (i d j) -> p i d j", i=39, d=4, j=39)
                gv = Gh[hi][:].rearrange("p (i j) -> p i j", i=39, j=39).unsqueeze(2).broadcast_to((128, 39, 4, 39))
                vv = (
                    v4[:, hi]
                    .transpose([0, 2, 1])
                    .unsqueeze(1)
                    .broadcast_to((128, 39, 4, 39))
                )
                nc.vector.tensor_tensor(out=P24, in0=vv, in1=gv, op=OP.mult)
                nc.vector.tensor_reduce(
                    ru4[:, :, hi, :], P24, axis=mybir.AxisListType.X, op=OP.add
                )
            hn = hpool.tile([128, 312], F32, tag="h")
            hn4 = hn[:].rearrange("p (i h d) -> p i h d", i=39, h=2, d=4)
            for hi in range(2):
                rv = rech[hi][:].unsqueeze(2).broadcast_to((128, 39, 4))
                nc.vector.tensor_tensor(
                    out=hn4[:, :, hi, :], in0=ru4[:, :, hi, :], in1=rv, op=OP.mult
                )
            nc.vector.tensor_tensor(out=hn[:], in0=hn[:], in1=rs[:], op=OP.add)
            nc.scalar.activation(hn[:], hn[:], AF.Relu)
            return hn

        def emit_head(ti, h_sb):
            s0 = ti * 128
            ht, pT, tms = emit_transposes(h_sb)
            lms = []
            for g in range(3):
                lm = nc.tensor.matmul(
                    pT[0:128, 384:386],
                    ht[:, 128 * g : 128 * (g + 1)],
                    cs["fcw"][:, 2 * g : 2 * (g + 1)],
                    start=(g == 0),
                    stop=(g == 2),
                )
                if lms:
                    _dep(lms[-1], lm, sync=False)
                lms.append(lm)
            la = nc.vector.tensor_tensor(
                out=logits[:, 2 * ti : 2 * ti + 2],
                in0=pT[0:128, 384:386],
                in1=cs["fcb"][:],
                op=OP.add,
            )
            _dep(lms[-1], la)
            nc.sync.dma_start(emb_d[s0 : s0 + 128, :], h_sb[:])

        # pairwise software pipeline: B's DVE attention hides A's PE/ACT
        # projection chain and vice versa
        assert n_tiles % 2 == 0
        n_pairs = n_tiles // 2
        hA = emit_embed(0)
        hB = emit_embed(1)
        for tp in range(n_pairs):
            A, B = 2 * tp, 2 * tp + 1
            hA_next = hB_next = None
            for li in range(NL):
                pjA = emit_proj(hA, li)
                pjB = emit_proj(hB, li)
                if li == 1 and tp + 1 < n_pairs:
                    # prefetch next pair's embeddings into the DVE stream
                    hA_next = emit_embed(2 * tp + 2)
                    hB_next = emit_embed(2 * tp + 3)
                hA = emit_att(*pjA)
                hB = emit_att(*pjB)
            emit_head(A, hA)
            emit_head(B, hB)
            hA, hB = hA_next, hB_next

        # ---------------- batched output softmax ----------------
        eL = smpool.tile([128, 2 * n_tiles], F32, tag="eL")
        nc.scalar.activation(eL[:], logits[:], AF.Exp)
        denL = smpool.tile([128, n_tiles], F32, tag="denL")
        nc.vector.tensor_reduce(
            denL[:],
            eL[:].rearrange("p (t c) -> p t c", c=2),
            axis=mybir.AxisListType.X,
            op=OP.add,
        )
        recL = smpool.tile([128, n_tiles], F32, tag="recL")
        nc.vector.reciprocal(recL[:], denL[:])
        oL = smpool.tile([128, 2 * n_tiles], F32, tag="oL")
        nc.vector.tensor_tensor(
            out=oL[:].rearrange("p (t c) -> p t c", c=2),
            in0=eL[:].rearrange("p (t c) -> p t c", c=2),
            in1=recL[:].unsqueeze(2).broadcast_to((128, n_tiles, 2)),
            op=OP.mult,
        )
        nc.sync.dma_start(outp_d[:], oL[:])

    nc.compile()
    return nc


_NC_CACHE = {}


def _get_nc(b_loc):
    if b_loc not in _NC_CACHE:
        _NC_CACHE[b_loc] = build_nc(b_loc)
    return _NC_CACHE[b_loc]


# ---------------------------------------------------------------- entry point
def kernel(
    x,
    emb_tables,
    dense_W,
    dense_b,
    Wq,
    Wk,
    Wv,
    Wres,
    fc_W,
    fc_b,
    _trace=False,
    _trace_kwargs=None,
):
    x = np.asarray(x, np.float32)
    B = x.shape[0]
    b_loc = B // N_CORES
    n_tiles = b_loc // 128
    consts = _build_consts(
        emb_tables, dense_W, dense_b, Wq, Wk, Wv, Wres, fc_W, fc_b
    )
    nc = _get_nc(b_loc)
    in_maps = []
    for i in range(N_CORES):
        shard = np.ascontiguousarray(x[i * b_loc : (i + 1) * b_loc].T)
        in_maps.append({"xt": shard, **consts})
    kw = {}
    if _trace:
        kw["trace"] = True
        if _trace_kwargs:
            kw.update(_trace_kwargs)
    res = run_bass_kernel_spmd(nc, in_maps, list(range(N_CORES)), **kw)
    embs = []
    outs = []
    for i in range(N_CORES):
        embs.append(res.results[i]["emb"])
        o = res.results[i]["outp"]
        outs.append(
            o.reshape(128, n_tiles, 2).transpose(1, 0, 2).reshape(b_loc, 2)
        )
    kernel._last_results = res
    return np.concatenate(embs, 0), np.concatenate(outs, 0)


# revision 28
# speedup vs baseline: 1.0207x; 1.0207x over previous
"""Trainium2 Bass kernel for nn_AutoInt (AutoInt CTR model).

Model: 26 sparse fields (one-hot embedding lookup, VOC=11) + 13 dense fields
(per-field Linear(1,8)) -> h [B, 39, 8]; 3 multi-head (H=2, D=4) interacting
layers (self-attention over the 39 feature fields, residual proj, relu);
outputs emb = h.reshape(B, 312) and out = softmax(emb @ fc_W + fc_b).

Strategy (pure data parallel over 8 cores, batch-sharded):
  - host: transpose each core's x shard to [39, b_loc]; build layout-only
    constants (block-diagonal weights, one-hot selector/compare tables).
  - device, per 128-sample tile (sample-major h [128, 312] throughout):
      embedding: replicate-rows matmul -> is_equal -> one-hot @ blockdiag
      tables (PE), dense part fused into the same PSUM bank.
      per layer: PE-transpose h -> h^T groups; projections with h^T as the
      matmul *stationary* and block-diag Wq|Wk|Wv|Wres as moving -> q,k,v,res
      sample-major; scores/softmax/att@v via DVE broadcast-view tensor ops +
      ACT exp; residual + relu.
      head: h^T groups @ fc_W accumulate -> logits; batched softmax at end.
"""

import os
from contextlib import ExitStack

import numpy as np

import concourse.bass as bass
import concourse.tile as tile
from concourse import bacc, mybir
from concourse.bass_utils import run_bass_kernel_spmd

F32 = mybir.dt.float32
AF = mybir.ActivationFunctionType
OP = mybir.AluOpType


def _dep(a, b, sync=True, reason="psum-group order"):
    """Order instruction b after a (BassInstruction wrappers)."""
    tile.add_dep_helper(b.ins, a.ins, sync=sync, reason=reason)

N_SPARSE, N_DENSE = 26, 13
F, E, H, D, NL, VOC = 39, 8, 2, 4, 3, 11
N_CORES = 8


# ---------------------------------------------------------------- host consts
def _build_consts(emb_tables, dense_W, dense_b, Wq, Wk, Wv, Wres, fc_W, fc_b):
    f32 = np.float32
    emb_tables = np.asarray(emb_tables, f32)
    dense_W = np.asarray(dense_W, f32)
    dense_b = np.asarray(dense_b, f32)
    fc_W = np.asarray(fc_W, f32)
    fc_b = np.asarray(fc_b, f32)

    # block-diag projection weights, all layers side by side: [104, 12*104]
    eye13 = np.eye(13, dtype=f32)
    wbd = np.concatenate(
        [
            np.kron(eye13, np.asarray(W[l], f32))
            for l in range(NL)
            for W in (Wq, Wk, Wv, Wres)
        ],
        axis=1,
    )

    # one-hot row-replication selector R [39, 287]
    R = np.zeros((39, 287), f32)
    for g in range(2):
        for fl in range(11):
            for v in range(VOC):
                R[g * 11 + fl, 121 * g + 11 * fl + v] = 1.0
    for fl in range(4):
        for v in range(VOC):
            R[22 + fl, 242 + 11 * fl + v] = 1.0

    # compare values [121, 2] (col 0: groups 0/1; col 1: group 2 + ones-row)
    vals = np.zeros((121, 2), f32)
    vals[:, 0] = np.tile(np.arange(VOC, dtype=f32), 11)
    vals[:44, 1] = np.tile(np.arange(VOC, dtype=f32), 4)
    # row 44 col 1 stays 0.0 -> is_equal(0, 0) = 1 = the "ones" row

    # embedding tables, block diagonal per group
    tbd01 = np.zeros((121, 176), f32)
    for g in range(2):
        for fl in range(11):
            tbd01[11 * fl : 11 * fl + 11, 88 * g + 8 * fl : 88 * g + 8 * fl + 8] = (
                emb_tables[g * 11 + fl]
            )
    tbd2e = np.zeros((45, 136), f32)
    for fl in range(4):
        tbd2e[11 * fl : 11 * fl + 11, 8 * fl : 8 * fl + 8] = emb_tables[22 + fl]
    tbd2e[44, 32:136] = dense_b.reshape(104)

    # dense weight diag [13, 104]
    wdd = np.zeros((13, 104), f32)
    for fl in range(13):
        wdd[fl, 8 * fl : 8 * fl + 8] = dense_W[fl]

    idn = np.eye(128, dtype=f32)
    fcw = np.ascontiguousarray(
        np.concatenate([fc_W.reshape(3, 104, 2)[g] for g in range(3)], axis=1)
    )  # [104, 6]
    fcb = np.ascontiguousarray(np.broadcast_to(fc_b.reshape(1, 2), (128, 2)))

    return {
        "wbd": np.ascontiguousarray(wbd),
        "rmat": R,
        "vals": vals,
        "tbd01": tbd01,
        "tbd2e": tbd2e,
        "wdd": wdd,
        "idn": idn,
        "fcw": fcw,
        "fcb": fcb,
    }


CONST_SHAPES = {
    "wbd": (104, 1248),
    "rmat": (39, 287),
    "vals": (121, 2),
    "tbd01": (121, 176),
    "tbd2e": (45, 136),
    "wdd": (13, 104),
    "idn": (128, 128),
    "fcw": (104, 6),
    "fcb": (128, 2),
}


# ---------------------------------------------------------------- device build
def build_nc(b_loc):
    n_tiles = b_loc // 128
    assert b_loc % 128 == 0
    nc = bacc.Bacc(
        "TRN2", target_bir_lowering=False, debug=False, num_devices=N_CORES
    )
    xt_d = nc.dram_tensor("xt", [39, b_loc], F32, kind="ExternalInput").ap()
    cd = {
        name: nc.dram_tensor(name, list(shape), F32, kind="ExternalInput").ap()
        for name, shape in CONST_SHAPES.items()
    }
    emb_d = nc.dram_tensor("emb", [b_loc, 312], F32, kind="ExternalOutput").ap()
    outp_d = nc.dram_tensor(
        "outp", [128, 2 * n_tiles], F32, kind="ExternalOutput"
    ).ap()

    with tile.TileContext(nc) as tc, ExitStack() as ctx:
        cpool = ctx.enter_context(tc.tile_pool(name="consts", bufs=1))
        cs = {}
        for name, shape in CONST_SHAPES.items():
            t = cpool.tile(list(shape), F32, tag=name)
            nc.sync.dma_start(t[:], cd[name][:])
            cs[name] = t
        xt = cpool.tile([39, b_loc], F32, tag="xt")
        nc.sync.dma_start(xt[:], xt_d[:])
        xdense = cpool.tile([13, b_loc], F32, tag="xdense")
        nc.sync.dma_start(xdense[:], xt_d[26:39, :])
        logits = cpool.tile([128, 2 * n_tiles], F32, tag="logits")

        hpool = ctx.enter_context(tc.tile_pool(name="h", bufs=6))
        htpool = ctx.enter_context(tc.tile_pool(name="ht", bufs=3))
        ohpool = ctx.enter_context(tc.tile_pool(name="oh", bufs=3))
        qkvr = ctx.enter_context(tc.tile_pool(name="qkvr", bufs=3))
        spool = ctx.enter_context(tc.tile_pool(name="sco", bufs=2))
        p2pool = ctx.enter_context(tc.tile_pool(name="p2", bufs=2))
        smpool = ctx.enter_context(tc.tile_pool(name="small", bufs=3))
        psA = ctx.enter_context(tc.tile_pool(name="psA", bufs=2, space="PSUM"))
        psB = ctx.enter_context(tc.tile_pool(name="psB", bufs=2, space="PSUM"))

        def emit_embed(ti):
            """One-hot embedding + dense features via PE -> h [128, 312]."""
            s0 = ti * 128
            pB = psB.tile([128, 512], F32, tag="psB")
            xcol = xt[:, s0 : s0 + 128]
            rmm1 = nc.tensor.matmul(
                pB[0:121, 0:128], cs["rmat"][:, 0:121], xcol, start=True, stop=False
            )
            rmm2 = nc.tensor.matmul(
                pB[0:45, 256:384], cs["rmat"][:, 242:287], xcol,
                start=False, stop=False,
            )
            rmm3 = nc.tensor.matmul(
                pB[0:121, 128:256], cs["rmat"][:, 121:242], xcol,
                start=False, stop=True,
            )
            _dep(rmm1, rmm2, sync=False)
            _dep(rmm2, rmm3, sync=False)
            oh = ohpool.tile([121, 384], F32)
            eqs = []
            for sl, vcol in (
                ((slice(0, 121), slice(0, 128)), 0),
                ((slice(0, 121), slice(128, 256)), 0),
                ((slice(0, 45), slice(256, 384)), 1),
            ):
                nr = sl[0].stop
                eq = nc.vector.tensor_scalar(
                    out=oh[sl[0], sl[1]],
                    in0=pB[sl[0], sl[1]],
                    scalar1=cs["vals"][0:nr, vcol : vcol + 1],
                    scalar2=None,
                    op0=OP.is_equal,
                )
                _dep(rmm3, eq)
                eqs.append(eq)
            pA = psA.tile([128, 1536], F32, tag="psA")
            em1 = nc.tensor.matmul(
                pA[:, 0:88], oh[0:121, 0:128], cs["tbd01"][:, 0:88],
                start=True, stop=False,
            )
            em2 = nc.tensor.matmul(
                pA[:, 88:176], oh[0:121, 128:256], cs["tbd01"][:, 88:176],
                start=False, stop=False,
            )
            em3 = nc.tensor.matmul(
                pA[:, 176:312], oh[0:45, 256:384], cs["tbd2e"][:],
                start=False, stop=False,
            )
            em4 = nc.tensor.matmul(
                pA[:, 208:312], xdense[:, s0 : s0 + 128], cs["wdd"][:],
                start=False, stop=True,
            )
            _dep(em1, em2, sync=False)
            _dep(em2, em3, sync=False)
            _dep(em3, em4, sync=False)
            h_sb = hpool.tile([128, 312], F32, tag="h")
            hcp = nc.scalar.copy(h_sb[:], pA[:, 0:312])
            _dep(em4, hcp)
            return h_sb

        def emit_transposes(h_sb):
            """PE-transpose h into h^T groups [104, 384] (PE + ACT only)."""
            pT = psB.tile([128, 512], F32, tag="psB")
            tms = []
            for g in range(3):
                tm = nc.tensor.matmul(
                    pT[0:104, 128 * g : 128 * (g + 1)],
                    h_sb[:, 104 * g : 104 * (g + 1)],
                    cs["idn"][:],
                    is_transpose=True,
                    start=(g == 0),
                    stop=(g == 2),
                )
                if tms:
                    _dep(tms[-1], tm, sync=False)
                tms.append(tm)
            ht = htpool.tile([104, 384], F32)
            for g in range(3):
                c = nc.scalar.copy(
                    ht[:, 128 * g : 128 * (g + 1)], pT[0:104, 128 * g : 128 * (g + 1)]
                )
                _dep(tms[-1], c)
            return ht, pT, tms

        def emit_proj(h_sb, li):
            """Projections q,k,v,res sample-major [128, 312] (PE + ACT only)."""
            ht, _, _ = emit_transposes(h_sb)
            pP = psA.tile([128, 1536], F32, tag="psA")
            for g in range(3):
                nc.tensor.matmul(
                    pP[:, 512 * g : 512 * g + 416],
                    ht[:, 128 * g : 128 * (g + 1)],
                    cs["wbd"][:, 416 * li : 416 * (li + 1)],
                    start=True,
                    stop=True,
                )
            qs = qkvr.tile([128, 312], F32, tag="q")
            ks = qkvr.tile([128, 312], F32, tag="k")
            vs = qkvr.tile([128, 312], F32, tag="v")
            rs = qkvr.tile([128, 312], F32, tag="r")
            pview = pP[:].rearrange("p (g c) -> p g c", g=3, c=512)
            for pi, dst in enumerate((qs, ks, vs, rs)):
                src = pview[:, :, 104 * pi : 104 * (pi + 1)]
                dstv = dst[:].rearrange("p (g c) -> p g c", g=3, c=104)
                nc.scalar.copy(dstv, src)
            return qs, ks, vs, rs

        def emit_att(qs, ks, vs, rs):
            """Scores/softmax/att@v/residual on DVE (+ ACT exp, relu)."""
            q4 = qs[:].rearrange("p (i h d) -> p h i d", i=39, h=2, d=4)
            k4 = ks[:].rearrange("p (j h d) -> p h j d", j=39, h=2, d=4)
            v4 = vs[:].rearrange("p (j h d) -> p h j d", j=39, h=2, d=4)
            Sh = []
            for hi in range(2):
                S = spool.tile([128, 1521], F32, tag=f"S{hi}")
                tmp = spool.tile([128, 1521], F32, tag=f"tmp{hi}")
                S3 = S[:].rearrange("p (i j) -> p i j", i=39, j=39)
                t3 = tmp[:].rearrange("p (i j) -> p i j", i=39, j=39)
                for di in range(4):
                    dst = S3 if di == 0 else t3
                    qv = q4[:, hi, :, di].unsqueeze(2).broadcast_to((128, 39, 39))
                    kv = k4[:, hi, :, di].unsqueeze(1).broadcast_to((128, 39, 39))
                    nc.vector.tensor_tensor(out=dst, in0=kv, in1=qv, op=OP.mult)
                    if di > 0:
                        nc.vector.tensor_tensor(
                            out=S[:], in0=S[:], in1=tmp[:], op=OP.add
                        )
                Sh.append(S)
            Gh, rech = [], []
            for hi in range(2):
                G = spool.tile([128, 1521], F32, tag=f"G{hi}")
                nc.scalar.activation(G[:], Sh[hi][:], AF.Exp)
                den = smpool.tile([128, 39], F32, tag=f"den{hi}")
                nc.vector.tensor_reduce(
                    den[:],
                    G[:].rearrange("p (i j) -> p i j", i=39, j=39),
                    axis=mybir.AxisListType.X,
                    op=OP.add,
                )
                rec = smpool.tile([128, 39], F32, tag=f"rec{hi}")
                nc.vector.reciprocal(rec[:], den[:])
                Gh.append(G)
                rech.append(rec)
            run_t = smpool.tile([128, 312], F32, tag="run")
            ru4 = run_t[:].rearrange("p (i h d) -> p i h d", i=39, h=2, d=4)
            for hi in range(2):
                P2 = p2pool.tile([128, 6084], F32, tag="P2")
                P24 = P2[:].rearrange("p (i d j) -> p i d j", i=39, d=4, j=39)
                gv = Gh[hi][:].rearrange("p (i j) -> p i j", i=39, j=39).unsqueeze(2).broadcast_to((128, 39, 4, 39))
                vv = (
                    v4[:, hi]
                    .transpose([0, 2, 1])
                    .unsqueeze(1)
                    .broadcast_to((128, 39, 4, 39))
                )
                nc.vector.tensor_tensor(out=P24, in0=vv, in1=gv, op=OP.mult)
                nc.vector.tensor_reduce(
                    ru4[:, :, hi, :], P24, axis=mybir.AxisListType.X, op=OP.add
                )
            hn = hpool.tile([128, 312], F32, tag="h")
            hn4 = hn[:].rearrange("p (i h d) -> p i h d", i=39, h=2, d=4)
            for hi in range(2):
                rv = rech[hi][:].unsqueeze(2).broadcast_to((128, 39, 4))
                nc.vector.tensor_tensor(
                    out=hn4[:, :, hi, :], in0=ru4[:, :, hi, :], in1=rv, op=OP.mult
                )
            nc.vector.tensor_tensor(out=hn[:], in0=hn[:], in1=rs[:], op=OP.add)
            nc.scalar.activation(hn[:], hn[:], AF.Relu)
            return hn

        def emit_head(ti, h_sb):
            s0 = ti * 128
            ht, pT, tms = emit_transposes(h_sb)
            lms = []
            for g in range(3):
                lm = nc.tensor.matmul(
                    pT[0:128, 384:386],
                    ht[:, 128 * g : 128 * (g + 1)],
                    cs["fcw"][:, 2 * g : 2 * (g + 1)],
                    start=(g == 0),
                    stop=(g == 2),
                )
                if lms:
                    _dep(lms[-1], lm, sync=False)
                lms.append(lm)
            la = nc.vector.tensor_tensor(
                out=logits[:, 2 * ti : 2 * ti + 2],
                in0=pT[0:128, 384:386],
                in1=cs["fcb"][:],
                op=OP.add,
            )
            _dep(lms[-1], la)
            nc.sync.dma_start(emb_d[s0 : s0 + 128, :], h_sb[:])

        # pairwise software pipeline: B's DVE attention hides A's PE/ACT
        # projection chain and vice versa
        assert n_tiles % 2 == 0
        n_pairs = n_tiles // 2
        hA = emit_embed(0)
        hB = emit_embed(1)
        pjA = emit_proj(hA, 0)
        pjB = emit_proj(hB, 0)
        for tp in range(n_pairs):
            A, B = 2 * tp, 2 * tp + 1
            hA_next = hB_next = None
            for li in range(NL):
                if li == 1 and tp + 1 < n_pairs:
                    # prefetch next pair's embeddings into the DVE stream
                    hA_next = emit_embed(2 * tp + 2)
                    hB_next = emit_embed(2 * tp + 3)
                hA2 = emit_att(*pjA)
                hB2 = emit_att(*pjB)
                if li < NL - 1:
                    pjA = emit_proj(hA2, li + 1)
                    pjB = emit_proj(hB2, li + 1)
                hA, hB = hA2, hB2
            if tp + 1 < n_pairs:
                # next pair's layer-0 projections ahead of the heads, so the
                # PE chain is done before the DVE drains the current pair
                pjA = emit_proj(hA_next, 0)
                pjB = emit_proj(hB_next, 0)
            emit_head(A, hA)
            emit_head(B, hB)
            hA, hB = hA_next, hB_next

        # ---------------- batched output softmax ----------------
        eL = smpool.tile([128, 2 * n_tiles], F32, tag="eL")
        nc.scalar.activation(eL[:], logits[:], AF.Exp)
        denL = smpool.tile([128, n_tiles], F32, tag="denL")
        nc.vector.tensor_reduce(
            denL[:],
            eL[:].rearrange("p (t c) -> p t c", c=2),
            axis=mybir.AxisListType.X,
            op=OP.add,
        )
        recL = smpool.tile([128, n_tiles], F32, tag="recL")
        nc.vector.reciprocal(recL[:], denL[:])
        oL = smpool.tile([128, 2 * n_tiles], F32, tag="oL")
        nc.vector.tensor_tensor(
            out=oL[:].rearrange("p (t c) -> p t c", c=2),
            in0=eL[:].rearrange("p (t c) -> p t c", c=2),
            in1=recL[:].unsqueeze(2).broadcast_to((128, n_tiles, 2)),
            op=OP.mult,
        )
        nc.sync.dma_start(outp_d[:], oL[:])

    nc.compile()
    return nc


_NC_CACHE = {}


def _get_nc(b_loc):
    if b_loc not in _NC_CACHE:
        _NC_CACHE[b_loc] = build_nc(b_loc)
    return _NC_CACHE[b_loc]


# ---------------------------------------------------------------- entry point
def kernel(
    x,
    emb_tables,
    dense_W,
    dense_b,
    Wq,
    Wk,
    Wv,
    Wres,
    fc_W,
    fc_b,
    _trace=False,
    _trace_kwargs=None,
):
    x = np.asarray(x, np.float32)
    B = x.shape[0]
    b_loc = B // N_CORES
    n_tiles = b_loc // 128
    consts = _build_consts(
        emb_tables, dense_W, dense_b, Wq, Wk, Wv, Wres, fc_W, fc_b
    )
    nc = _get_nc(b_loc)
    in_maps = []
    for i in range(N_CORES):
        shard = np.ascontiguousarray(x[i * b_loc : (i + 1) * b_loc].T)
        in_maps.append({"xt": shard, **consts})
    kw = {}
    if _trace:
        kw["trace"] = True
        if _trace_kwargs:
            kw.update(_trace_kwargs)
    res = run_bass_kernel_spmd(nc, in_maps, list(range(N_CORES)), **kw)
    embs = []
    outs = []
    for i in range(N_CORES):
        embs.append(res.results[i]["emb"])
        o = res.results[i]["outp"]
        outs.append(
            o.reshape(128, n_tiles, 2).transpose(1, 0, 2).reshape(b_loc, 2)
        )
    kernel._last_results = res
    return np.concatenate(embs, 0), np.concatenate(outs, 0)


# revision 29
# speedup vs baseline: 1.0215x; 1.0007x over previous
"""Trainium2 Bass kernel for nn_AutoInt (AutoInt CTR model).

Model: 26 sparse fields (one-hot embedding lookup, VOC=11) + 13 dense fields
(per-field Linear(1,8)) -> h [B, 39, 8]; 3 multi-head (H=2, D=4) interacting
layers (self-attention over the 39 feature fields, residual proj, relu);
outputs emb = h.reshape(B, 312) and out = softmax(emb @ fc_W + fc_b).

Strategy (pure data parallel over 8 cores, batch-sharded):
  - host: transpose each core's x shard to [39, b_loc]; build layout-only
    constants (block-diagonal weights, one-hot selector/compare tables).
  - device, per 128-sample tile (sample-major h [128, 312] throughout):
      embedding: replicate-rows matmul -> is_equal -> one-hot @ blockdiag
      tables (PE), dense part fused into the same PSUM bank.
      per layer: PE-transpose h -> h^T groups; projections with h^T as the
      matmul *stationary* and block-diag Wq|Wk|Wv|Wres as moving -> q,k,v,res
      sample-major; scores/softmax/att@v via DVE broadcast-view tensor ops +
      ACT exp; residual + relu.
      head: h^T groups @ fc_W accumulate -> logits; batched softmax at end.
"""

import os
from contextlib import ExitStack

import numpy as np

import concourse.bass as bass
import concourse.tile as tile
from concourse import bacc, mybir
from concourse.bass_utils import run_bass_kernel_spmd

F32 = mybir.dt.float32
AF = mybir.ActivationFunctionType
OP = mybir.AluOpType


def _dep(a, b, sync=True, reason="psum-group order"):
    """Order instruction b after a (BassInstruction wrappers)."""
    tile.add_dep_helper(b.ins, a.ins, sync=sync, reason=reason)

N_SPARSE, N_DENSE = 26, 13
F, E, H, D, NL, VOC = 39, 8, 2, 4, 3, 11
N_CORES = 8


# ---------------------------------------------------------------- host consts
def _build_consts(emb_tables, dense_W, dense_b, Wq, Wk, Wv, Wres, fc_W, fc_b):
    f32 = np.float32
    emb_tables = np.asarray(emb_tables, f32)
    dense_W = np.asarray(dense_W, f32)
    dense_b = np.asarray(dense_b, f32)
    fc_W = np.asarray(fc_W, f32)
    fc_b = np.asarray(fc_b, f32)

    # block-diag projection weights, all layers side by side: [104, 12*104]
    eye13 = np.eye(13, dtype=f32)
    wbd = np.concatenate(
        [
            np.kron(eye13, np.asarray(W[l], f32))
            for l in range(NL)
            for W in (Wq, Wk, Wv, Wres)
        ],
        axis=1,
    )

    # one-hot row-replication selector R [39, 287]
    R = np.zeros((39, 287), f32)
    for g in range(2):
        for fl in range(11):
            for v in range(VOC):
                R[g * 11 + fl, 121 * g + 11 * fl + v] = 1.0
    for fl in range(4):
        for v in range(VOC):
            R[22 + fl, 242 + 11 * fl + v] = 1.0

    # compare values [121, 2] (col 0: groups 0/1; col 1: group 2 + ones-row)
    vals = np.zeros((121, 2), f32)
    vals[:, 0] = np.tile(np.arange(VOC, dtype=f32), 11)
    vals[:44, 1] = np.tile(np.arange(VOC, dtype=f32), 4)
    # row 44 col 1 stays 0.0 -> is_equal(0, 0) = 1 = the "ones" row

    # embedding tables, block diagonal per group
    tbd01 = np.zeros((121, 176), f32)
    for g in range(2):
        for fl in range(11):
            tbd01[11 * fl : 11 * fl + 11, 88 * g + 8 * fl : 88 * g + 8 * fl + 8] = (
                emb_tables[g * 11 + fl]
            )
    tbd2e = np.zeros((45, 136), f32)
    for fl in range(4):
        tbd2e[11 * fl : 11 * fl + 11, 8 * fl : 8 * fl + 8] = emb_tables[22 + fl]
    tbd2e[44, 32:136] = dense_b.reshape(104)

    # dense weight diag [13, 104]
    wdd = np.zeros((13, 104), f32)
    for fl in range(13):
        wdd[fl, 8 * fl : 8 * fl + 8] = dense_W[fl]

    idn = np.eye(128, dtype=f32)
    fcw = np.ascontiguousarray(
        np.concatenate([fc_W.reshape(3, 104, 2)[g] for g in range(3)], axis=1)
    )  # [104, 6]
    fcb = np.ascontiguousarray(np.broadcast_to(fc_b.reshape(1, 2), (128, 2)))

    return {
        "wbd": np.ascontiguousarray(wbd),
        "rmat": R,
        "vals": vals,
        "tbd01": tbd01,
        "tbd2e": tbd2e,
        "wdd": wdd,
        "idn": idn,
        "fcw": fcw,
        "fcb": fcb,
    }


CONST_SHAPES = {
    "wbd": (104, 1248),
    "rmat": (39, 287),
    "vals": (121, 2),
    "tbd01": (121, 176),
    "tbd2e": (45, 136),
    "wdd": (13, 104),
    "idn": (128, 128),
    "fcw": (104, 6),
    "fcb": (128, 2),
}


# ---------------------------------------------------------------- device build
def build_nc(b_loc):
    n_tiles = b_loc // 128
    assert b_loc % 128 == 0
    nc = bacc.Bacc(
        "TRN2", target_bir_lowering=False, debug=False, num_devices=N_CORES
    )
    xt_d = nc.dram_tensor("xt", [39, b_loc], F32, kind="ExternalInput").ap()
    cd = {
        name: nc.dram_tensor(name, list(shape), F32, kind="ExternalInput").ap()
        for name, shape in CONST_SHAPES.items()
    }
    emb_d = nc.dram_tensor("emb", [b_loc, 312], F32, kind="ExternalOutput").ap()
    outp_d = nc.dram_tensor(
        "outp", [128, 2 * n_tiles], F32, kind="ExternalOutput"
    ).ap()

    with tile.TileContext(nc) as tc, ExitStack() as ctx:
        cpool = ctx.enter_context(tc.tile_pool(name="consts", bufs=1))
        # embedding-critical tensors first on the sync HWDGE ring; bulky
        # constants go via the scalar-issued ring so they load in parallel
        xt = cpool.tile([39, b_loc], F32, tag="xt")
        nc.sync.dma_start(xt[:], xt_d[:])
        cs = {}
        early = ("rmat", "vals", "tbd01", "tbd2e", "wdd")
        for name in early:
            t = cpool.tile(list(CONST_SHAPES[name]), F32, tag=name)
            nc.sync.dma_start(t[:], cd[name][:])
            cs[name] = t
        for name, shape in CONST_SHAPES.items():
            if name in early:
                continue
            t = cpool.tile(list(shape), F32, tag=name)
            nc.scalar.dma_start(t[:], cd[name][:])
            cs[name] = t
        xdense = cpool.tile([13, b_loc], F32, tag="xdense")
        nc.scalar.dma_start(xdense[:], xt_d[26:39, :])
        logits = cpool.tile([128, 2 * n_tiles], F32, tag="logits")

        hpool = ctx.enter_context(tc.tile_pool(name="h", bufs=6))
        htpool = ctx.enter_context(tc.tile_pool(name="ht", bufs=3))
        ohpool = ctx.enter_context(tc.tile_pool(name="oh", bufs=3))
        qkvr = ctx.enter_context(tc.tile_pool(name="qkvr", bufs=3))
        spool = ctx.enter_context(tc.tile_pool(name="sco", bufs=2))
        p2pool = ctx.enter_context(tc.tile_pool(name="p2", bufs=2))
        smpool = ctx.enter_context(tc.tile_pool(name="small", bufs=3))
        psA = ctx.enter_context(tc.tile_pool(name="psA", bufs=2, space="PSUM"))
        psB = ctx.enter_context(tc.tile_pool(name="psB", bufs=2, space="PSUM"))

        def emit_embed(ti):
            """One-hot embedding + dense features via PE -> h [128, 312]."""
            s0 = ti * 128
            pB = psB.tile([128, 512], F32, tag="psB")
            xcol = xt[:, s0 : s0 + 128]
            rmm1 = nc.tensor.matmul(
                pB[0:121, 0:128], cs["rmat"][:, 0:121], xcol, start=True, stop=False
            )
            rmm2 = nc.tensor.matmul(
                pB[0:45, 256:384], cs["rmat"][:, 242:287], xcol,
                start=False, stop=False,
            )
            rmm3 = nc.tensor.matmul(
                pB[0:121, 128:256], cs["rmat"][:, 121:242], xcol,
                start=False, stop=True,
            )
            _dep(rmm1, rmm2, sync=False)
            _dep(rmm2, rmm3, sync=False)
            oh = ohpool.tile([121, 384], F32)
            eqs = []
            for sl, vcol in (
                ((slice(0, 121), slice(0, 128)), 0),
                ((slice(0, 121), slice(128, 256)), 0),
                ((slice(0, 45), slice(256, 384)), 1),
            ):
                nr = sl[0].stop
                eq = nc.vector.tensor_scalar(
                    out=oh[sl[0], sl[1]],
                    in0=pB[sl[0], sl[1]],
                    scalar1=cs["vals"][0:nr, vcol : vcol + 1],
                    scalar2=None,
                    op0=OP.is_equal,
                )
                _dep(rmm3, eq)
                eqs.append(eq)
            pA = psA.tile([128, 1536], F32, tag="psA")
            em1 = nc.tensor.matmul(
                pA[:, 0:88], oh[0:121, 0:128], cs["tbd01"][:, 0:88],
                start=True, stop=False,
            )
            em2 = nc.tensor.matmul(
                pA[:, 88:176], oh[0:121, 128:256], cs["tbd01"][:, 88:176],
                start=False, stop=False,
            )
            em3 = nc.tensor.matmul(
                pA[:, 176:312], oh[0:45, 256:384], cs["tbd2e"][:],
                start=False, stop=False,
            )
            em4 = nc.tensor.matmul(
                pA[:, 208:312], xdense[:, s0 : s0 + 128], cs["wdd"][:],
                start=False, stop=True,
            )
            _dep(em1, em2, sync=False)
            _dep(em2, em3, sync=False)
            _dep(em3, em4, sync=False)
            h_sb = hpool.tile([128, 312], F32, tag="h")
            hcp = nc.scalar.copy(h_sb[:], pA[:, 0:312])
            _dep(em4, hcp)
            return h_sb

        def emit_transposes(h_sb):
            """PE-transpose h into h^T groups [104, 384] (PE + ACT only)."""
            pT = psB.tile([128, 512], F32, tag="psB")
            tms = []
            for g in range(3):
                tm = nc.tensor.matmul(
                    pT[0:104, 128 * g : 128 * (g + 1)],
                    h_sb[:, 104 * g : 104 * (g + 1)],
                    cs["idn"][:],
                    is_transpose=True,
                    start=(g == 0),
                    stop=(g == 2),
                )
                if tms:
                    _dep(tms[-1], tm, sync=False)
                tms.append(tm)
            ht = htpool.tile([104, 384], F32)
            for g in range(3):
                c = nc.scalar.copy(
                    ht[:, 128 * g : 128 * (g + 1)], pT[0:104, 128 * g : 128 * (g + 1)]
                )
                _dep(tms[-1], c)
            return ht, pT, tms

        def emit_proj(h_sb, li):
            """Projections q,k,v,res sample-major [128, 312] (PE + ACT only)."""
            ht, _, _ = emit_transposes(h_sb)
            pP = psA.tile([128, 1536], F32, tag="psA")
            for g in range(3):
                nc.tensor.matmul(
                    pP[:, 512 * g : 512 * g + 416],
                    ht[:, 128 * g : 128 * (g + 1)],
                    cs["wbd"][:, 416 * li : 416 * (li + 1)],
                    start=True,
                    stop=True,
                )
            qs = qkvr.tile([128, 312], F32, tag="q")
            ks = qkvr.tile([128, 312], F32, tag="k")
            vs = qkvr.tile([128, 312], F32, tag="v")
            rs = qkvr.tile([128, 312], F32, tag="r")
            pview = pP[:].rearrange("p (g c) -> p g c", g=3, c=512)
            for pi, dst in enumerate((qs, ks, vs, rs)):
                src = pview[:, :, 104 * pi : 104 * (pi + 1)]
                dstv = dst[:].rearrange("p (g c) -> p g c", g=3, c=104)
                nc.scalar.copy(dstv, src)
            return qs, ks, vs, rs

        def emit_att(qs, ks, vs, rs):
            """Scores/softmax/att@v/residual on DVE (+ ACT exp, relu)."""
            q4 = qs[:].rearrange("p (i h d) -> p h i d", i=39, h=2, d=4)
            k4 = ks[:].rearrange("p (j h d) -> p h j d", j=39, h=2, d=4)
            v4 = vs[:].rearrange("p (j h d) -> p h j d", j=39, h=2, d=4)
            Sh = []
            for hi in range(2):
                S = spool.tile([128, 1521], F32, tag=f"S{hi}")
                tmp = spool.tile([128, 1521], F32, tag=f"tmp{hi}")
                S3 = S[:].rearrange("p (i j) -> p i j", i=39, j=39)
                t3 = tmp[:].rearrange("p (i j) -> p i j", i=39, j=39)
                for di in range(4):
                    dst = S3 if di == 0 else t3
                    qv = q4[:, hi, :, di].unsqueeze(2).broadcast_to((128, 39, 39))
                    kv = k4[:, hi, :, di].unsqueeze(1).broadcast_to((128, 39, 39))
                    nc.vector.tensor_tensor(out=dst, in0=kv, in1=qv, op=OP.mult)
                    if di > 0:
                        nc.vector.tensor_tensor(
                            out=S[:], in0=S[:], in1=tmp[:], op=OP.add
                        )
                Sh.append(S)
            Gh, rech = [], []
            for hi in range(2):
                G = spool.tile([128, 1521], F32, tag=f"G{hi}")
                nc.scalar.activation(G[:], Sh[hi][:], AF.Exp)
                den = smpool.tile([128, 39], F32, tag=f"den{hi}")
                nc.vector.tensor_reduce(
                    den[:],
                    G[:].rearrange("p (i j) -> p i j", i=39, j=39),
                    axis=mybir.AxisListType.X,
                    op=OP.add,
                )
                rec = smpool.tile([128, 39], F32, tag=f"rec{hi}")
                nc.vector.reciprocal(rec[:], den[:])
                Gh.append(G)
                rech.append(rec)
            run_t = smpool.tile([128, 312], F32, tag="run")
            ru4 = run_t[:].rearrange("p (i h d) -> p i h d", i=39, h=2, d=4)
            for hi in range(2):
                P2 = p2pool.tile([128, 6084], F32, tag="P2")
                P24 = P2[:].rearrange("p (i d j) -> p i d j", i=39, d=4, j=39)
                gv = Gh[hi][:].rearrange("p (i j) -> p i j", i=39, j=39).unsqueeze(2).broadcast_to((128, 39, 4, 39))
                vv = (
                    v4[:, hi]
                    .transpose([0, 2, 1])
                    .unsqueeze(1)
                    .broadcast_to((128, 39, 4, 39))
                )
                nc.vector.tensor_tensor(out=P24, in0=vv, in1=gv, op=OP.mult)
                nc.vector.tensor_reduce(
                    ru4[:, :, hi, :], P24, axis=mybir.AxisListType.X, op=OP.add
                )
            hn = hpool.tile([128, 312], F32, tag="h")
            hn4 = hn[:].rearrange("p (i h d) -> p i h d", i=39, h=2, d=4)
            for hi in range(2):
                rv = rech[hi][:].unsqueeze(2).broadcast_to((128, 39, 4))
                nc.vector.tensor_tensor(
                    out=hn4[:, :, hi, :], in0=ru4[:, :, hi, :], in1=rv, op=OP.mult
                )
            nc.vector.tensor_tensor(out=hn[:], in0=hn[:], in1=rs[:], op=OP.add)
            nc.scalar.activation(hn[:], hn[:], AF.Relu)
            return hn

        def emit_head(ti, h_sb):
            s0 = ti * 128
            ht, pT, tms = emit_transposes(h_sb)
            lms = []
            for g in range(3):
                lm = nc.tensor.matmul(
                    pT[0:128, 384:386],
                    ht[:, 128 * g : 128 * (g + 1)],
                    cs["fcw"][:, 2 * g : 2 * (g + 1)],
                    start=(g == 0),
                    stop=(g == 2),
                )
                if lms:
                    _dep(lms[-1], lm, sync=False)
                lms.append(lm)
            la = nc.vector.tensor_tensor(
                out=logits[:, 2 * ti : 2 * ti + 2],
                in0=pT[0:128, 384:386],
                in1=cs["fcb"][:],
                op=OP.add,
            )
            _dep(lms[-1], la)
            nc.sync.dma_start(emb_d[s0 : s0 + 128, :], h_sb[:])

        # pairwise software pipeline: B's DVE attention hides A's PE/ACT
        # projection chain and vice versa
        assert n_tiles % 2 == 0
        n_pairs = n_tiles // 2
        hA = emit_embed(0)
        hB = emit_embed(1)
        pjA = emit_proj(hA, 0)
        pjB = emit_proj(hB, 0)
        for tp in range(n_pairs):
            A, B = 2 * tp, 2 * tp + 1
            hA_next = hB_next = None
            for li in range(NL):
                if li == 1 and tp + 1 < n_pairs:
                    # prefetch next pair's embeddings into the DVE stream
                    hA_next = emit_embed(2 * tp + 2)
                    hB_next = emit_embed(2 * tp + 3)
                hA2 = emit_att(*pjA)
                hB2 = emit_att(*pjB)
                if li < NL - 1:
                    pjA = emit_proj(hA2, li + 1)
                    pjB = emit_proj(hB2, li + 1)
                hA, hB = hA2, hB2
            if tp + 1 < n_pairs:
                # next pair's layer-0 projections ahead of the heads, so the
                # PE chain is done before the DVE drains the current pair
                pjA = emit_proj(hA_next, 0)
                pjB = emit_proj(hB_next, 0)
            emit_head(A, hA)
            emit_head(B, hB)
            hA, hB = hA_next, hB_next

        # ---------------- batched output softmax ----------------
        eL = smpool.tile([128, 2 * n_tiles], F32, tag="eL")
        nc.scalar.activation(eL[:], logits[:], AF.Exp)
        denL = smpool.tile([128, n_tiles], F32, tag="denL")
        nc.vector.tensor_reduce(
            denL[:],
            eL[:].rearrange("p (t c) -> p t c", c=2),
            axis=mybir.AxisListType.X,
            op=OP.add,
        )
        recL = smpool.tile([128, n_tiles], F32, tag="recL")
        nc.vector.reciprocal(recL[:], denL[:])
        oL = smpool.tile([128, 2 * n_tiles], F32, tag="oL")
        nc.vector.tensor_tensor(
            out=oL[:].rearrange("p (t c) -> p t c", c=2),
            in0=eL[:].rearrange("p (t c) -> p t c", c=2),
            in1=recL[:].unsqueeze(2).broadcast_to((128, n_tiles, 2)),
            op=OP.mult,
        )
        nc.sync.dma_start(outp_d[:], oL[:])

    nc.compile()
    return nc


_NC_CACHE = {}


def _get_nc(b_loc):
    if b_loc not in _NC_CACHE:
        _NC_CACHE[b_loc] = build_nc(b_loc)
    return _NC_CACHE[b_loc]


# ---------------------------------------------------------------- entry point
def kernel(
    x,
    emb_tables,
    dense_W,
    dense_b,
    Wq,
    Wk,
    Wv,
    Wres,
    fc_W,
    fc_b,
    _trace=False,
    _trace_kwargs=None,
):
    x = np.asarray(x, np.float32)
    B = x.shape[0]
    b_loc = B // N_CORES
    n_tiles = b_loc // 128
    consts = _build_consts(
        emb_tables, dense_W, dense_b, Wq, Wk, Wv, Wres, fc_W, fc_b
    )
    nc = _get_nc(b_loc)
    in_maps = []
    for i in range(N_CORES):
        shard = np.ascontiguousarray(x[i * b_loc : (i + 1) * b_loc].T)
        in_maps.append({"xt": shard, **consts})
    kw = {}
    if _trace:
        kw["trace"] = True
        if _trace_kwargs:
            kw.update(_trace_kwargs)
    res = run_bass_kernel_spmd(nc, in_maps, list(range(N_CORES)), **kw)
    embs = []
    outs = []
    for i in range(N_CORES):
        embs.append(res.results[i]["emb"])
        o = res.results[i]["outp"]
        outs.append(
            o.reshape(128, n_tiles, 2).transpose(1, 0, 2).reshape(b_loc, 2)
        )
    kernel._last_results = res
    return np.concatenate(embs, 0), np.concatenate(outs, 0)
